# revision 27
# baseline (speedup 1.0000x reference)
"""Trainium2 Bass kernel for nn_PairwisePredictionHead.

Math (reference):
  xd = x @ W_down.T + b_down             # [L, 128]
  q, k = xd[:, :64], xd[:, 64:]
  h[i,j,:] = W1p @ (q_j*k_i) + W1d @ (q_j - k_i) + b1    # [L, L, 128]
  g = gelu_exact(h)
  out = W2 @ LN(g) + b2                   # [L, L, 64]

Sharding: row-shard i across 8 cores (96 rows each). Each core gets the full
q-side (all 768 j) plus its own 96 k-rows; cores are independent (no
collectives), outputs concatenated on host.

Per-core device algorithm (layout: h on partitions, pairs on free):
  - lhsT_i = [[W1p.T * k_i[:,None]] ; W1d.T]  (bf16; only top half per i)
  - psum1[h, j] = lhsT_i.T @ [q.T; q.T]            (PE bf16, N=768)
  - g = Gelu(psum1 + (b1 - W1d@k_i))               (ACT, bf16 out)
  - g2 = g*g                                       (DVE)
  - per 128-j chunk c: po[j, 66c:66c+65] = (g_c stationary) @ [W2z.T | 1/128]
                       po[j, 66c+65]    = (g2_c stationary) @ [1/128]
    W2z = (W2*ln_g) - rowmean: zero-mean rows absorb LN's mean subtraction.
    The 1/128 columns make po's extra cols mu and E[g^2] directly.
  - per 2 rows: mu2 = mu*mu; veps = (E[g2]+eps) - mu2; r = veps^-1/2 (gpsimd)
  - per row: o = bf16(po * r)  (DVE, fp32 PSUM read)
             o2 = o[main cols] + c_full  (DVE bf16 2x; c = W2@ln_b + b2)
  - DMA o2 (bf16) to HBM; host casts fp32.
"""

import os
from contextlib import ExitStack

import numpy as np
import ml_dtypes

import concourse.bass as bass
import concourse.mybir as mybir
import concourse.tile as tile
from concourse import bacc
from concourse.bass_utils import run_bass_kernel_spmd

F32 = mybir.dt.float32
BF16 = mybir.dt.bfloat16
ALU = mybir.AluOpType
AF = mybir.ActivationFunctionType

LAST_RES = None

B, L, D = 1, 768, 1024
DP, H, NB = 128, 128, 64
NCORES = 8
ROWS = L // NCORES  # 96 pair-grid rows per core
P = 128
EPS = 1e-5


def _build(nc):
    xT = nc.dram_tensor("xT", [P, 8, L], BF16, kind="ExternalInput")
    xTr = nc.dram_tensor("xTr", [P, 8, ROWS], BF16, kind="ExternalInput")
    WdTq = nc.dram_tensor("WdTq", [P, 8, 64], BF16, kind="ExternalInput")
    WdTk = nc.dram_tensor("WdTk", [P, 8, 64], BF16, kind="ExternalInput")
    bdq = nc.dram_tensor("bdq", [64, 1], F32, kind="ExternalInput")
    bdk = nc.dram_tensor("bdk", [64, 1], F32, kind="ExternalInput")
    W1pT = nc.dram_tensor("W1pT", [64, P], BF16, kind="ExternalInput")
    W1dT = nc.dram_tensor("W1dT", [64, P], BF16, kind="ExternalInput")
    b1v = nc.dram_tensor("b1v", [P, 1], F32, kind="ExternalInput")
    W2ze = nc.dram_tensor("W2ze", [P, 65], BF16, kind="ExternalInput")
    cfull = nc.dram_tensor("cfull", [P, 384], BF16, kind="ExternalInput")
    out = nc.dram_tensor("out", [ROWS, L, NB], BF16, kind="ExternalOutput")

    with tile.TileContext(nc) as tc, ExitStack() as ctx:
        const = ctx.enter_context(tc.tile_pool(name="const", bufs=1))
        work = ctx.enter_context(tc.tile_pool(name="work", bufs=3))
        outp = ctx.enter_context(tc.tile_pool(name="outp", bufs=3))
        statsp = ctx.enter_context(tc.tile_pool(name="statsp", bufs=2))
        pp1 = ctx.enter_context(tc.tile_pool(name="pp1", bufs=2, space="PSUM"))
        ppo = ctx.enter_context(tc.tile_pool(name="ppo", bufs=1, space="PSUM"))

        # ---- constants into SBUF (host pre-swizzled, contiguous DMAs) ----
        xT_sb = const.tile([P, 8, L], BF16)
        for c in range(8):
            nc.sync.dma_start(out=xT_sb[:, c, :], in_=xT[:, c, :])
        xTr_sb = const.tile([P, 8, ROWS], BF16)
        nc.sync.dma_start(out=xTr_sb, in_=xTr[:])
        WdTq_sb = const.tile([P, 8, 64], BF16)
        nc.sync.dma_start(out=WdTq_sb, in_=WdTq[:])
        WdTk_sb = const.tile([P, 8, 64], BF16)
        nc.sync.dma_start(out=WdTk_sb, in_=WdTk[:])
        bdq_sb = const.tile([64, 1], F32)
        nc.sync.dma_start(out=bdq_sb, in_=bdq[:])
        bdk_sb = const.tile([64, 1], F32)
        nc.sync.dma_start(out=bdk_sb, in_=bdk[:])
        W1pT_sb = const.tile([64, P], BF16)
        nc.sync.dma_start(out=W1pT_sb, in_=W1pT[:])
        W1dT_sb = const.tile([64, P], BF16)
        nc.sync.dma_start(out=W1dT_sb, in_=W1dT[:])
        b1v_sb = const.tile([P, 1], F32)
        nc.sync.dma_start(out=b1v_sb, in_=b1v[:])
        W2ze_sb = const.tile([P, 65], BF16)
        nc.sync.dma_start(out=W2ze_sb, in_=W2ze[:])
        cfull_sb = const.tile([P, 6, 64], BF16)
        nc.sync.dma_start(out=cfull_sb, in_=cfull[:].rearrange("p (c w) -> p c w", w=64))
        mhalf = const.tile([P, 12], F32)
        nc.vector.memset(mhalf, -0.5)

        # ---- prep: qq = [q.T; q.T] (bf16), kT, b1c = b1 - W1d@kT ----
        qq = const.tile([P, L], BF16)
        kT_sb = const.tile([64, ROWS], F32)
        kTb_sb = const.tile([64, ROWS], BF16)
        b1c = const.tile([P, ROWS], F32)

        pq = pp1.tile([64, L], F32, tag="p1")
        for c in range(8):
            for h0, h1 in ((0, 512), (512, 768)):
                nc.tensor.matmul(
                    pq[:, h0:h1], WdTq_sb[:, c, :], xT_sb[:, c, h0:h1],
                    start=(c == 0), stop=(c == 7),
                )
        nc.scalar.activation(qq[0:64, :], pq, AF.Identity, bias=bdq_sb)
        nc.sync.dma_start(out=qq[64:128, :], in_=qq[0:64, :])

        pk = pp1.tile([64, ROWS], F32, tag="p1")
        for c in range(8):
            nc.tensor.matmul(pk, WdTk_sb[:, c, :], xTr_sb[:, c, :],
                             start=(c == 0), stop=(c == 7))
        nc.scalar.activation(kT_sb, pk, AF.Identity, bias=bdk_sb)
        nc.vector.tensor_copy(kTb_sb, kT_sb)

        # persistent W1 stationary tiles (bottom halves static = W1d.T)
        lhsT_t = [const.tile([P, P], BF16, tag=f"lhsT{t}", name=f"lhsT{t}")
                  for t in range(2)]
        for t in range(2):
            nc.sync.dma_start(out=lhsT_t[t][64:128, :], in_=W1dT[:])

        pc = pp1.tile([P, ROWS], F32, tag="p1")
        nc.tensor.matmul(pc, W1dT_sb, kTb_sb, start=True, stop=True)
        nc.scalar.activation(b1c, pc, AF.Identity, bias=b1v_sb, scale=-1.0)

        # 4-bank rotating PSUM block for po; row ii uses bank ii % 4.
        po_blk = ppo.tile([P, 4, 512], F32)
        r_bufs = [None, None]

        # ---- main loop ----
        # Software-pipelined: mm1(ii+1) is emitted BEFORE mm2(ii) so the PE
        # FIFO never idles while ACT/DVE produce g/g2 for mm2(ii).
        def emit_mm1(ii):
            lt = lhsT_t[ii % 2]
            nc.vector.tensor_scalar_mul(lt[0:64, :], W1pT_sb, kT_sb[:, ii:ii + 1])
            p1 = pp1.tile([P, L], F32, tag="p1", name="p1")
            nc.tensor.matmul(p1[:, 0:512], lt, qq[:, 0:512], start=True, stop=True)
            nc.tensor.matmul(p1[:, 512:768], lt, qq[:, 512:768],
                             start=True, stop=True)
            return p1

        def emit_scale(row):
            """osb = bf16(po[row] * r). Gates PSUM bank recycling; r was
            computed 2 blocks earlier so this never waits on the pow."""
            b = row // 2
            r_sb = r_bufs[b % 2]
            k = row % 2
            po_r = po_blk[:, row % 4, 0:396].rearrange("p (c w) -> p c w", w=66)
            osb = outp.tile([P, 6, 64], BF16, tag="osb", name="osb")
            rb = r_sb[:, k * 6:k * 6 + 6, None].broadcast_to([P, 6, 64])
            nc.vector.tensor_mul(osb, po_r[:, :, 0:64], rb)
            return osb

        def emit_cadd_dma(row, osb):
            o2 = outp.tile([P, 384], BF16, tag="o2", name="o2")
            nc.vector.tensor_tensor(
                o2, osb[:].rearrange("p c w -> p (c w)"),
                cfull_sb[:].rearrange("p c w -> p (c w)"), ALU.add)
            nc.sync.dma_start(
                out=out[row].rearrange("(c p) n -> p c n", p=P),
                in_=o2[:].rearrange("p (c n) -> p c n", n=NB))

        def emit_stats(b):
            """Stats + pow for batch b (rows 2b, 2b+1); emitted one block
            after mm2(2b+1), so nothing here blocks the gelu cadence."""
            s0 = 2 * b
            pv = po_blk[:, (s0 % 4):(s0 % 4) + 2, 0:396]
            pvr = pv.rearrange("p b (c w) -> p b c w", w=66)
            mu2 = statsp.tile([P, 2, 6], F32, tag="mu2", name="mu2")
            nc.scalar.activation(mu2, pvr[:, :, :, 64], AF.Square)
            veps = statsp.tile([P, 12], F32, tag="veps", name="veps")
            nc.vector.scalar_tensor_tensor(
                veps[:].rearrange("p (b c) -> p b c", b=2),
                pvr[:, :, :, 65], EPS, mu2[:], ALU.add, ALU.subtract)
            r_bufs[b % 2] = r_sb = statsp.tile(
                [P, 12], F32, tag="r", name="r")
            nc.gpsimd.tensor_tensor(r_sb, veps[:], mhalf[:], ALU.pow)

        p1_bufs = [None, None]
        p1_bufs[0] = emit_mm1(0)
        for ii in range(ROWS):
            p1 = p1_bufs[ii % 2]
            if ii + 1 < ROWS:
                p1_bufs[(ii + 1) % 2] = emit_mm1(ii + 1)
            g = work.tile([P, L], BF16, tag="g", name="g")
            nc.scalar.activation(g, p1, AF.Gelu, bias=b1c[:, ii:ii + 1])

            osbs = None
            if ii % 2 == 0 and ii >= 4:
                # scales run on DVE while ACT does the gelu above
                osbs = [emit_scale(ii - 4), emit_scale(ii - 3)]

            g2 = work.tile([P, L], BF16, tag="g2", name="g2")
            if ii % 4 == 1:
                nc.scalar.square(g2, g)
            else:
                nc.vector.tensor_mul(g2, g, g)

            if osbs is not None:
                emit_cadd_dma(ii - 4, osbs[0])
                emit_cadd_dma(ii - 3, osbs[1])

            po = po_blk[:, ii % 4, 0:396]
            for c in range(6):
                nc.tensor.matmul(po[:, c * 66:c * 66 + 65],
                                 g[:, c * 128:(c + 1) * 128], W2ze_sb,
                                 start=(c == 0), stop=False)
                nc.tensor.matmul(po[:, c * 66 + 65:c * 66 + 66],
                                 g2[:, c * 128:(c + 1) * 128],
                                 W2ze_sb[:, 64:65],
                                 start=False, stop=(c == 5))

            if ii % 2 == 0 and ii >= 2:
                emit_stats((ii - 2) // 2)

        emit_stats(ROWS // 2 - 1)
        for row in range(ROWS - 4, ROWS):
            osb = emit_scale(row)
            emit_cadd_dma(row, osb)


def host_prep(x, W_down, b_down, W1, b1, ln_g, ln_b, W2, b2):
    f32 = np.float32
    bf16 = ml_dtypes.bfloat16
    def swz(a):  # [1024, M] -> [128, 8, M] with row c*128+p -> [p, c]
        return np.ascontiguousarray(
            np.asarray(a, dtype=np.float32).reshape(8, P, -1)
            .transpose(1, 0, 2).astype(bf16))

    xTfull = np.ascontiguousarray(x[0].T.astype(f32))  # [D, L]
    common = {
        "xT": swz(xTfull),
        "WdTq": swz(W_down[:64, :].T),
        "WdTk": swz(W_down[64:, :].T),
        "bdq": np.ascontiguousarray(b_down[:64].astype(f32).reshape(64, 1)),
        "bdk": np.ascontiguousarray(b_down[64:].astype(f32).reshape(64, 1)),
        "W1pT": np.ascontiguousarray(W1[:, :64].T.astype(bf16)),
        "W1dT": np.ascontiguousarray(W1[:, 64:].T.astype(bf16)),
        "b1v": np.ascontiguousarray(b1.astype(f32).reshape(P, 1)),
    }
    W2g = W2.astype(np.float64) * ln_g.astype(np.float64)[None, :]
    W2z = W2g - W2g.mean(axis=1, keepdims=True)
    W2ze = np.concatenate([W2z.T, np.full((P, 1), 1.0 / 128.0)], axis=1)
    common["W2ze"] = np.ascontiguousarray(W2ze.astype(bf16))
    cvec = W2.astype(np.float64) @ ln_b.astype(np.float64) + b2.astype(np.float64)
    common["cfull"] = np.ascontiguousarray(
        np.tile(cvec[None, :], (P, 6)).astype(bf16))
    return common, xTfull


def kernel(x, W_down, b_down, W1, b1, ln_g, ln_b, W2, b2):
    x = np.asarray(x)
    common, xTfull = host_prep(
        x, np.asarray(W_down), np.asarray(b_down), np.asarray(W1),
        np.asarray(b1), np.asarray(ln_g), np.asarray(ln_b), np.asarray(W2),
        np.asarray(b2))

    nc = bacc.Bacc("TRN2")
    _build(nc)
    nc.finalize()

    in_maps = []
    for core in range(NCORES):
        m = dict(common)
        i0 = core * ROWS
        m["xTr"] = np.ascontiguousarray(
            xTfull[:, i0:i0 + ROWS].reshape(8, P, ROWS).transpose(1, 0, 2)
            .astype(ml_dtypes.bfloat16))
        in_maps.append(m)

    trace = os.environ.get("KERNEL_TRACE", "0") == "1"
    res = run_bass_kernel_spmd(nc, in_maps, core_ids=list(range(NCORES)),
                               trace=trace)
    global LAST_RES
    LAST_RES = res
    if trace and res.exec_time_ns is not None:
        print(f"HW exec time: {res.exec_time_ns} ns")
    outs = [res.results[c]["out"] for c in range(NCORES)]
    full = np.concatenate(outs, axis=0)  # [768, 768, 64]
    return full[None].astype(np.float32)


# revision 28
# speedup vs baseline: 1.3643x; 1.3643x over previous
"""Trainium2 Bass kernel for nn_PairwisePredictionHead.

Math (reference):
  xd = x @ W_down.T + b_down             # [L, 128]
  q, k = xd[:, :64], xd[:, 64:]
  h[i,j,:] = W1p @ (q_j*k_i) + W1d @ (q_j - k_i) + b1    # [L, L, 128]
  g = gelu_exact(h)
  out = W2 @ LN(g) + b2                   # [L, L, 64]

Sharding: row-shard i across 8 cores (96 rows each). Each core gets the full
q-side (all 768 j) plus its own 96 k-rows; cores are independent (no
collectives), outputs concatenated on host.

Per-core device algorithm (layout: h on partitions, pairs on free):
  - lhsT_i = [[W1p.T * k_i[:,None]] ; W1d.T]  (bf16; only top half per i)
  - psum1[h, j] = lhsT_i.T @ [q.T; q.T]            (PE bf16, N=768)
  - g = Gelu(psum1 + (b1 - W1d@k_i))               (ACT, bf16 out)
  - g2 = g*g                                       (DVE)
  - per 128-j chunk c: po[j, 66c:66c+65] = (g_c stationary) @ [W2z.T | 1/128]
                       po[j, 66c+65]    = (g2_c stationary) @ [1/128]
    W2z = (W2*ln_g) - rowmean: zero-mean rows absorb LN's mean subtraction.
    The 1/128 columns make po's extra cols mu and E[g^2] directly.
  - per 2 rows: mu2 = mu*mu; veps = (E[g2]+eps) - mu2; r = veps^-1/2 (gpsimd)
  - per row: o = bf16(po * r)  (DVE, fp32 PSUM read)
             o2 = o[main cols] + c_full  (DVE bf16 2x; c = W2@ln_b + b2)
  - DMA o2 (bf16) to HBM; host casts fp32.
"""

import os
from contextlib import ExitStack

import numpy as np
import ml_dtypes

import concourse.bass as bass
import concourse.mybir as mybir
import concourse.tile as tile
from concourse import bacc
from concourse.bass_utils import run_bass_kernel_spmd

F32 = mybir.dt.float32
BF16 = mybir.dt.bfloat16
ALU = mybir.AluOpType
AF = mybir.ActivationFunctionType

LAST_RES = None

B, L, D = 1, 768, 1024
DP, H, NB = 128, 128, 64
NCORES = 8
ROWS = L // NCORES  # 96 pair-grid rows per core
P = 128
EPS = 1e-5


def _build(nc):
    xT = nc.dram_tensor("xT", [P, 8, L], BF16, kind="ExternalInput")
    xTr = nc.dram_tensor("xTr", [P, 8, ROWS], BF16, kind="ExternalInput")
    WdTq = nc.dram_tensor("WdTq", [P, 8, 64], BF16, kind="ExternalInput")
    WdTk = nc.dram_tensor("WdTk", [P, 8, 64], BF16, kind="ExternalInput")
    bdq = nc.dram_tensor("bdq", [64, 1], F32, kind="ExternalInput")
    bdk = nc.dram_tensor("bdk", [64, 1], F32, kind="ExternalInput")
    W1pT = nc.dram_tensor("W1pT", [64, P], BF16, kind="ExternalInput")
    W1dT = nc.dram_tensor("W1dT", [64, P], BF16, kind="ExternalInput")
    b1v = nc.dram_tensor("b1v", [P, 1], F32, kind="ExternalInput")
    W2ze = nc.dram_tensor("W2ze", [P, 65], BF16, kind="ExternalInput")
    cfull = nc.dram_tensor("cfull", [P, 384], BF16, kind="ExternalInput")
    out = nc.dram_tensor("out", [ROWS, L, NB], BF16, kind="ExternalOutput")

    with tile.TileContext(nc) as tc, ExitStack() as ctx:
        const = ctx.enter_context(tc.tile_pool(name="const", bufs=1))
        work = ctx.enter_context(tc.tile_pool(name="work", bufs=3))
        outp = ctx.enter_context(tc.tile_pool(name="outp", bufs=3))
        statsp = ctx.enter_context(tc.tile_pool(name="statsp", bufs=2))
        pp1 = ctx.enter_context(tc.tile_pool(name="pp1", bufs=2, space="PSUM"))
        ppo = ctx.enter_context(tc.tile_pool(name="ppo", bufs=1, space="PSUM"))

        # ---- constants into SBUF (host pre-swizzled, contiguous DMAs) ----
        xT_sb = const.tile([P, 8, L], BF16)
        for c in range(8):
            nc.sync.dma_start(out=xT_sb[:, c, :], in_=xT[:, c, :])
        xTr_sb = const.tile([P, 8, ROWS], BF16)
        nc.sync.dma_start(out=xTr_sb, in_=xTr[:])
        WdTq_sb = const.tile([P, 8, 64], BF16)
        nc.sync.dma_start(out=WdTq_sb, in_=WdTq[:])
        WdTk_sb = const.tile([P, 8, 64], BF16)
        nc.sync.dma_start(out=WdTk_sb, in_=WdTk[:])
        bdq_sb = const.tile([64, 1], F32)
        nc.sync.dma_start(out=bdq_sb, in_=bdq[:])
        bdk_sb = const.tile([64, 1], F32)
        nc.sync.dma_start(out=bdk_sb, in_=bdk[:])
        W1pT_sb = const.tile([64, P], BF16)
        nc.sync.dma_start(out=W1pT_sb, in_=W1pT[:])
        W1dT_sb = const.tile([64, P], BF16)
        nc.sync.dma_start(out=W1dT_sb, in_=W1dT[:])
        b1v_sb = const.tile([P, 1], F32)
        nc.sync.dma_start(out=b1v_sb, in_=b1v[:])
        W2ze_sb = const.tile([P, 65], BF16)
        nc.sync.dma_start(out=W2ze_sb, in_=W2ze[:])
        cfull_sb = const.tile([P, 6, 64], BF16)
        nc.sync.dma_start(out=cfull_sb, in_=cfull[:].rearrange("p (c w) -> p c w", w=64))
        mhalf = const.tile([P, 12], F32)
        nc.vector.memset(mhalf, -0.5)

        # ---- prep: qq = [q.T; q.T] (bf16), kT, b1c = b1 - W1d@kT ----
        qq = const.tile([P, L], BF16)
        kT_sb = const.tile([64, ROWS], F32)
        kTb_sb = const.tile([64, ROWS], BF16)
        b1c = const.tile([P, ROWS], F32)

        pq = pp1.tile([64, L], F32, tag="p1")
        for c in range(8):
            for h0, h1 in ((0, 512), (512, 768)):
                nc.tensor.matmul(
                    pq[:, h0:h1], WdTq_sb[:, c, :], xT_sb[:, c, h0:h1],
                    start=(c == 0), stop=(c == 7),
                )
        nc.scalar.activation(qq[0:64, :], pq, AF.Identity, bias=bdq_sb)
        nc.sync.dma_start(out=qq[64:128, :], in_=qq[0:64, :])

        pk = pp1.tile([64, ROWS], F32, tag="p1")
        for c in range(8):
            nc.tensor.matmul(pk, WdTk_sb[:, c, :], xTr_sb[:, c, :],
                             start=(c == 0), stop=(c == 7))
        nc.scalar.activation(kT_sb, pk, AF.Identity, bias=bdk_sb)
        nc.vector.tensor_copy(kTb_sb, kT_sb)

        # persistent W1 stationary tiles (bottom halves static = W1d.T)
        lhsT_t = [const.tile([P, P], BF16, tag=f"lhsT{t}", name=f"lhsT{t}")
                  for t in range(2)]
        for t in range(2):
            nc.sync.dma_start(out=lhsT_t[t][64:128, :], in_=W1dT[:])

        pc = pp1.tile([P, ROWS], F32, tag="p1")
        nc.tensor.matmul(pc, W1dT_sb, kTb_sb, start=True, stop=True)
        nc.scalar.activation(b1c, pc, AF.Identity, bias=b1v_sb, scale=-1.0)

        # 4-bank rotating PSUM block for po; row ii uses bank ii % 4.
        po_blk = ppo.tile([P, 4, 512], F32)
        r_bufs = [None, None]

        # ---- main loop ----
        # Software-pipelined: mm1(ii+1) is emitted BEFORE mm2(ii) so the PE
        # FIFO never idles while ACT/DVE produce g/g2 for mm2(ii).
        def emit_mm1(ii):
            lt = lhsT_t[ii % 2]
            nc.vector.tensor_scalar_mul(lt[0:64, :], W1pT_sb, kT_sb[:, ii:ii + 1])
            p1 = pp1.tile([P, L], F32, tag="p1", name="p1")
            nc.tensor.matmul(p1[:, 0:512], lt, qq[:, 0:512], start=True, stop=True)
            nc.tensor.matmul(p1[:, 512:768], lt, qq[:, 512:768],
                             start=True, stop=True)
            return p1

        def emit_out(row):
            """Scale row by r (ready >=1 block earlier: never waits the pow),
            add c, DMA. osb = bf16(po[row] * r) gates PSUM bank recycling."""
            b = row // 2
            r_sb = r_bufs[b % 2]
            k = row % 2
            po_r = po_blk[:, row % 4, 0:396].rearrange("p (c w) -> p c w", w=66)
            osb = outp.tile([P, 6, 64], BF16, tag="osb", name="osb")
            rb = r_sb[:, k * 6:k * 6 + 6, None].broadcast_to([P, 6, 64])
            nc.vector.tensor_mul(osb, po_r[:, :, 0:64], rb)
            o2 = outp.tile([P, 384], BF16, tag="o2", name="o2")
            nc.vector.tensor_tensor(
                o2, osb[:].rearrange("p c w -> p (c w)"),
                cfull_sb[:].rearrange("p c w -> p (c w)"), ALU.add)
            nc.sync.dma_start(
                out=out[row].rearrange("(c p) n -> p c n", p=P),
                in_=o2[:].rearrange("p (c n) -> p c n", n=NB))

        p1_bufs = [None, None]
        p1_bufs[0] = emit_mm1(0)
        for ii in range(ROWS):
            p1 = p1_bufs[ii % 2]
            if ii + 1 < ROWS:
                p1_bufs[(ii + 1) % 2] = emit_mm1(ii + 1)
            g = work.tile([P, L], BF16, tag="g", name="g")
            nc.scalar.activation(g, p1, AF.Gelu, bias=b1c[:, ii:ii + 1])
            g2 = work.tile([P, L], BF16, tag="g2", name="g2")
            if ii % 4 == 1:
                nc.scalar.square(g2, g)
            else:
                nc.vector.tensor_mul(g2, g, g)

            po = po_blk[:, ii % 4, 0:396]
            for c in range(6):
                nc.tensor.matmul(po[:, c * 66:c * 66 + 65],
                                 g[:, c * 128:(c + 1) * 128], W2ze_sb,
                                 start=(c == 0), stop=False)
                nc.tensor.matmul(po[:, c * 66 + 65:c * 66 + 66],
                                 g2[:, c * 128:(c + 1) * 128],
                                 W2ze_sb[:, 64:65],
                                 start=False, stop=(c == 5))

            if ii % 2 == 1:
                # stats for batch b = (rows ii-1, ii); all on DVE/gpsimd so
                # the ACT queue stays pure gelus.
                s0 = ii - 1
                pv = po_blk[:, (s0 % 4):(s0 % 4) + 2, 0:396]
                pvr = pv.rearrange("p b (c w) -> p b c w", w=66)
                stage = statsp.tile([P, 2, 6, 2], F32, tag="stage",
                                    name="stage")
                nc.vector.tensor_copy(stage, pvr[:, :, :, 64:66])
                stage_f = stage[:].rearrange("p b c two -> p (b c) two")
                mu2 = statsp.tile([P, 12], F32, tag="mu2", name="mu2")
                nc.vector.tensor_tensor(mu2, stage_f[:, :, 0],
                                        stage_f[:, :, 0], ALU.mult)
                veps = statsp.tile([P, 12], F32, tag="veps", name="veps")
                nc.vector.scalar_tensor_tensor(veps, stage_f[:, :, 1], EPS,
                                               mu2[:], ALU.add, ALU.subtract)
                r_bufs[(ii // 2) % 2] = r_sb = statsp.tile(
                    [P, 12], F32, tag="r", name="r")
                nc.gpsimd.tensor_tensor(r_sb, veps[:], mhalf[:], ALU.pow)

                if ii >= 3:
                    emit_out(ii - 3)
                    emit_out(ii - 2)

        for row in range(ROWS - 2, ROWS):
            emit_out(row)


def host_prep(x, W_down, b_down, W1, b1, ln_g, ln_b, W2, b2):
    f32 = np.float32
    bf16 = ml_dtypes.bfloat16
    def swz(a):  # [1024, M] -> [128, 8, M] with row c*128+p -> [p, c]
        return np.ascontiguousarray(
            np.asarray(a, dtype=np.float32).reshape(8, P, -1)
            .transpose(1, 0, 2).astype(bf16))

    xTfull = np.ascontiguousarray(x[0].T.astype(f32))  # [D, L]
    common = {
        "xT": swz(xTfull),
        "WdTq": swz(W_down[:64, :].T),
        "WdTk": swz(W_down[64:, :].T),
        "bdq": np.ascontiguousarray(b_down[:64].astype(f32).reshape(64, 1)),
        "bdk": np.ascontiguousarray(b_down[64:].astype(f32).reshape(64, 1)),
        "W1pT": np.ascontiguousarray(W1[:, :64].T.astype(bf16)),
        "W1dT": np.ascontiguousarray(W1[:, 64:].T.astype(bf16)),
        "b1v": np.ascontiguousarray(b1.astype(f32).reshape(P, 1)),
    }
    W2g = W2.astype(np.float64) * ln_g.astype(np.float64)[None, :]
    W2z = W2g - W2g.mean(axis=1, keepdims=True)
    W2ze = np.concatenate([W2z.T, np.full((P, 1), 1.0 / 128.0)], axis=1)
    common["W2ze"] = np.ascontiguousarray(W2ze.astype(bf16))
    cvec = W2.astype(np.float64) @ ln_b.astype(np.float64) + b2.astype(np.float64)
    common["cfull"] = np.ascontiguousarray(
        np.tile(cvec[None, :], (P, 6)).astype(bf16))
    return common, xTfull


def kernel(x, W_down, b_down, W1, b1, ln_g, ln_b, W2, b2):
    x = np.asarray(x)
    common, xTfull = host_prep(
        x, np.asarray(W_down), np.asarray(b_down), np.asarray(W1),
        np.asarray(b1), np.asarray(ln_g), np.asarray(ln_b), np.asarray(W2),
        np.asarray(b2))

    nc = bacc.Bacc("TRN2")
    _build(nc)
    nc.finalize()

    in_maps = []
    for core in range(NCORES):
        m = dict(common)
        i0 = core * ROWS
        m["xTr"] = np.ascontiguousarray(
            xTfull[:, i0:i0 + ROWS].reshape(8, P, ROWS).transpose(1, 0, 2)
            .astype(ml_dtypes.bfloat16))
        in_maps.append(m)

    trace = os.environ.get("KERNEL_TRACE", "0") == "1"
    res = run_bass_kernel_spmd(nc, in_maps, core_ids=list(range(NCORES)),
                               trace=trace)
    global LAST_RES
    LAST_RES = res
    if trace and res.exec_time_ns is not None:
        print(f"HW exec time: {res.exec_time_ns} ns")
    outs = [res.results[c]["out"] for c in range(NCORES)]
    full = np.concatenate(outs, axis=0)  # [768, 768, 64]
    return full[None].astype(np.float32)
